# revision 1
# baseline (speedup 1.0000x reference)
"""Trainium2 Bass kernel for a dense transformer encoder layer.

Problem (hardcoded): x [2, 2048, 1024], 16 heads, FFN 4096, fp32,
post-LN residual blocks, mask additively applied before softmax.

Sharding: sequence-parallel over the 4096 tokens -> 512 tokens per core
(cores 0-3 handle batch 0, cores 4-7 batch 1). Every core computes the
full-batch K/V projections itself (no collectives -- they are broken on
this stack), bounces K^T / V through internal DRAM, then runs attention
for its own 512 queries, the output projection, LN1, the FFN and LN2.

Matmul layouts (out = lhsT.T @ rhs, contraction on the partition dim):
  Q^T/K^T : lhsT = W k/m-tile [din,dout], rhs = x^T [din,tok]  -> [dout,tok]
  V       : lhsT = x^T [din,tok],  rhs = Wv [din,dout]         -> [tok,dout]
  scoresT : lhsT = K^T head [dh,kpos], rhs = Q^T head [dh,qpos]-> [kpos,qpos]
  attn@V' : lhsT = V' [kpos,dh+1], rhs = expT [kpos,qpos]      -> [dh+1,qpos]
            (V' has a ones column -> row dh is the softmax denominator)
  outproj : lhsT = o^T [din,q], rhs = Wp [din,dout]            -> [q,dout]
  FFN1    : lhsT = W1 [din,dffn], rhs = xln1^T [din,q]         -> [dffn,q]
  FFN2    : lhsT = h^T [dffn,q], rhs = W2 [dffn,dout]          -> [q,dout]

All matmuls run as float32r (1 cycle/row at free-dim >= 256).
"""

import numpy as np

import concourse.bass as bass
import concourse.mybir as mybir
import concourse.tile as tile
from concourse.bass_utils import run_bass_kernel_spmd
from concourse.masks import make_identity
from concourse.vector_clock import ScopedClock

FP32 = mybir.dt.float32
FP32R = mybir.dt.float32r
AF = mybir.ActivationFunctionType
ALU = mybir.AluOpType

P = 128
D = 1024
F = 4096
H = 16
DH = 64
S = 2048          # tokens per batch
TPC = 512         # tokens (queries) per core
NB = D // P       # 8 dout blocks
KB = D // P       # 8 contraction tiles over D
FB = F // P       # 32 dffn tiles
QT = TPC // P     # 4 query tiles
KT16 = S // P     # 16 kpos tiles
NG = S // TPC     # 4 kpos 512-slices
VW = H * (DH + 1)  # 1040: V' row width
SCALE = DH ** -0.5
EPS = 1e-6
N_CORES = 8


# --- Tile tail-drain fix: this walrus build allows only one sem-wait per
# instruction; Tile's final drain accumulates several. Split them across
# dedicated nops before draining.
def _patched_drain_and_barrier(self, tick_clock, wait_clock):
    probe = self.nc.sync.nop(nofuse=True, hint="drain_wait_split")
    wait_clock.add_sem_waits(probe.ins, ScopedClock({None: tick_clock.global_clock}))
    si = probe.ins.sync_info
    if si is not None and si.on_wait and len(si.on_wait) > 1:
        waits = list(si.on_wait)
        si.on_wait = waits[:1]
        for w in waits[1:]:
            extra = self.nc.sync.nop(nofuse=True, hint="drain_wait_split")
            esi = extra.ins.sync_info
            if esi is None:
                extra.ins.sync_info = mybir.SyncInfo(on_wait=[w], on_update=[])
            else:
                esi.on_wait = [w]
    self.nc.sync.drain()
    self.nc.all_engine_barrier()
    assert self.sems is not None
    popped = self.nc._tile_sem_poison_stack.pop()
    assert popped is self._sem_poison
    self.nc.clear_and_free_semaphores(list(self.sems.allocated().values()))
    self.nc.all_engine_barrier()


if getattr(tile.TileContext, "_drain_patch", None) is None:
    tile.TileContext._drain_and_barrier = _patched_drain_and_barrier
    tile.TileContext._drain_patch = True


def _r(ap):
    return ap.bitcast(FP32R)


def _split_waits(nc):
    """Walrus codegen accepts at most one sem-wait per instruction (two on
    EventSemaphore). Tile's scheduler can emit more; hoist the surplus onto
    same-engine EventSemaphore instructions inserted just before."""
    uid = [0]
    for bb in nc.m.functions[0].blocks:
        new_insts = []
        for inst in bb.instructions:
            si = inst.sync_info
            limit = 2 if isinstance(inst, mybir.InstEventSemaphore) else 1
            if si is not None and si.on_wait and len(si.on_wait) > limit:
                waits = list(si.on_wait)
                extra, keep = waits[:-limit], waits[-limit:]
                for i in range(0, len(extra), 2):
                    uid[0] += 1
                    ev = mybir.InstEventSemaphore(
                        name=f"I-wsplit-{uid[0]}",
                        engine=inst.engine,
                        sync_info=mybir.SyncInfo(
                            on_wait=extra[i:i + 2], on_update=[]),
                    )
                    nc.register_instruction(ev)
                    new_insts.append(ev)
                si.on_wait = keep
            new_insts.append(inst)
        if len(new_insts) != len(bb.instructions):
            bb.instructions[:] = new_insts


def _ln_chain(nc, pool, y, out_ap, gamma_b, beta_b, eps_t):
    """LayerNorm over the free dim of y [128, D] (torch semantics:
    unbiased std, denominator std + eps), writing to out_ap."""
    s1 = pool.tile([P, 1], FP32, tag="ln_s1")
    nc.vector.reduce_sum(s1[:], y[:], axis=mybir.AxisListType.X)
    mn = pool.tile([P, 1], FP32, tag="ln_mn")
    nc.scalar.mul(mn[:], s1[:], 1.0 / D)
    cen = pool.tile([P, D], FP32, tag="ln_cen")
    nc.vector.tensor_scalar_sub(cen[:], y[:], mn[:])
    sq = pool.tile([P, D], FP32, tag="ln_sq")
    ss = pool.tile([P, 1], FP32, tag="ln_ss")
    nc.scalar.activation(sq[:], cen[:], AF.Square, accum_out=ss[:])
    var = pool.tile([P, 1], FP32, tag="ln_var")
    nc.scalar.mul(var[:], ss[:], 1.0 / (D - 1))
    std = pool.tile([P, 1], FP32, tag="ln_std")
    nc.scalar.activation(std[:], var[:], AF.Sqrt)
    nc.scalar.activation(std[:], std[:], AF.Identity, bias=eps_t[:])
    inv = pool.tile([P, 1], FP32, tag="ln_inv")
    nc.vector.reciprocal(inv[:], std[:])
    nc.vector.scalar_tensor_tensor(
        out_ap, cen[:], inv[:], gamma_b[:], op0=ALU.mult, op1=ALU.mult
    )
    nc.vector.tensor_add(out_ap, out_ap, beta_b[:])


def build_program(use_mask: bool) -> bass.Bass:
    nc = bass.Bass(target_bir_lowering=False, debug=False)

    # ---- I/O ----
    xT_d = nc.dram_tensor("xT", [D, S], FP32, kind="ExternalInput")
    xTq_d = nc.dram_tensor("xTq", [D, TPC], FP32, kind="ExternalInput")
    xblk_d = nc.dram_tensor("xblk", [TPC, D], FP32, kind="ExternalInput")
    wq_d = nc.dram_tensor("wq", [D, D], FP32, kind="ExternalInput")
    wk_d = nc.dram_tensor("wk", [D, D], FP32, kind="ExternalInput")
    wv_d = nc.dram_tensor("wv", [D, D], FP32, kind="ExternalInput")
    wp_d = nc.dram_tensor("wp", [D, D], FP32, kind="ExternalInput")
    w1_d = nc.dram_tensor("w1", [D, F], FP32, kind="ExternalInput")
    w2_d = nc.dram_tensor("w2", [F, D], FP32, kind="ExternalInput")
    bq_d = nc.dram_tensor("bq", [D], FP32, kind="ExternalInput")
    bk_d = nc.dram_tensor("bk", [D], FP32, kind="ExternalInput")
    bv_d = nc.dram_tensor("bv", [D], FP32, kind="ExternalInput")
    bp_d = nc.dram_tensor("bp", [D], FP32, kind="ExternalInput")
    b1_d = nc.dram_tensor("b1", [F], FP32, kind="ExternalInput")
    b2_d = nc.dram_tensor("b2", [D], FP32, kind="ExternalInput")
    g1_d = nc.dram_tensor("g1", [D], FP32, kind="ExternalInput")
    be1_d = nc.dram_tensor("be1", [D], FP32, kind="ExternalInput")
    g2_d = nc.dram_tensor("g2", [D], FP32, kind="ExternalInput")
    be2_d = nc.dram_tensor("be2", [D], FP32, kind="ExternalInput")
    if use_mask:
        maskT_d = nc.dram_tensor("maskT", [S, TPC], FP32, kind="ExternalInput")
    out_d = nc.dram_tensor("out", [TPC, D], FP32, kind="ExternalOutput")

    with tile.TileContext(nc) as tc:
        with tc.tile_pool(name="dram", bufs=1, space="DRAM") as dramp:
            KT_t = [dramp.tile([P, S], FP32, name=f"KT_{m}") for m in range(NB)]
            V_t = [dramp.tile([P, D], FP32, name=f"V_{mt}") for mt in range(KT16)]
            _build_body(
                nc, tc, use_mask,
                xT_d, xTq_d, xblk_d, wq_d, wk_d, wv_d, wp_d, w1_d, w2_d,
                bq_d, bk_d, bv_d, bp_d, b1_d, b2_d, g1_d, be1_d, g2_d, be2_d,
                maskT_d if use_mask else None, out_d, KT_t, V_t,
            )
    _split_waits(nc)
    return nc


def _build_body(nc, tc, use_mask, xT_d, xTq_d, xblk_d, wq_d, wk_d, wv_d,
                wp_d, w1_d, w2_d, bq_d, bk_d, bv_d, bp_d, b1_d, b2_d,
                g1_d, be1_d, g2_d, be2_d, maskT_d, out_d, KT_t, V_t):
    from contextlib import ExitStack

    with ExitStack() as top:
        consts = top.enter_context(tc.tile_pool(name="consts", bufs=1))
        ident = consts.tile([P, P], FP32)
        make_identity(nc, ident[:])
        bq_c = consts.tile([P, NB], FP32)
        nc.sync.dma_start(bq_c[:], bq_d.ap().rearrange("(b p) -> p b", p=P))
        bk_c = consts.tile([P, NB], FP32)
        nc.sync.dma_start(bk_c[:], bk_d.ap().rearrange("(b p) -> p b", p=P))
        bv_c = consts.tile([P, NB], FP32)
        nc.sync.dma_start(bv_c[:], bv_d.ap().rearrange("(b p) -> p b", p=P))
        b1_c = consts.tile([P, FB], FP32)
        nc.sync.dma_start(b1_c[:], b1_d.ap().rearrange("(b p) -> p b", p=P))
        eps_t = consts.tile([P, 1], FP32)
        nc.vector.memset(eps_t[:], EPS)
        ones16 = consts.tile([P, H], FP32)
        nc.vector.memset(ones16[:], 1.0)
        ones_raw = consts.tile([1, DH], FP32)
        nc.vector.memset(ones_raw[:], 1.0)
        ones_row = consts.tile([1, DH], FP32)
        nc.scalar.copy(_r(ones_row[:]), ones_raw[:])

        persist = top.enter_context(tc.tile_pool(name="persist", bufs=1))
        qt_sb = persist.tile([P, NB * TPC], FP32)   # Q^T, 16KB/part
        ot_sb = persist.tile([P, NB * TPC], FP32)   # o^T, 16KB/part

        # ================= QKV phase =================
        with (
            tc.tile_pool(name="qkv_sb", bufs=1) as qp,
            tc.tile_pool(name="qkv_w", bufs=2) as wpool,
            tc.tile_pool(name="qkv_stage", bufs=6) as stg,
            tc.tile_pool(name="qkv_ps", bufs=8, space="PSUM") as pp,
        ):
            xt = qp.tile([P, KB * S], FP32)          # x^T full batch, 64KB
            for b in range(KB):
                nc.sync.dma_start(_r(xt[:, b * S:(b + 1) * S]),
                                  _r(xT_d.ap()[b * P:(b + 1) * P, :]))
            xtq = qp.tile([P, KB * TPC], FP32)       # own x^T columns, 16KB
            for b in range(KB):
                nc.sync.dma_start(_r(xtq[:, b * TPC:(b + 1) * TPC]),
                                  _r(xTq_d.ap()[b * P:(b + 1) * P, :]))

            # --- Q^T -> qt_sb (+bq) ---
            w_sb = wpool.tile([P, KB * D], FP32, tag="wfull")   # 32KB
            for b in range(KB):
                nc.sync.dma_start(_r(w_sb[:, b * D:(b + 1) * D]),
                                  _r(wq_d.ap()[b * P:(b + 1) * P, :]))
            for m in range(NB):
                ps = pp.tile([P, TPC], FP32, tag="qkvps")
                for k in range(KB):
                    nc.tensor.matmul(
                        ps[:],
                        lhsT=_r(w_sb[:, k * D + m * P: k * D + (m + 1) * P]),
                        rhs=_r(xtq[:, k * TPC:(k + 1) * TPC]),
                        start=(k == 0), stop=(k == KB - 1),
                    )
                nc.vector.tensor_scalar_add(
                    _r(qt_sb[:, m * TPC:(m + 1) * TPC]), ps[:], bq_c[:, m:m + 1])

            # --- K^T -> DRAM (+bk) ---
            w_sb = wpool.tile([P, KB * D], FP32, tag="wfull")
            for b in range(KB):
                nc.sync.dma_start(_r(w_sb[:, b * D:(b + 1) * D]),
                                  _r(wk_d.ap()[b * P:(b + 1) * P, :]))
            for ng in range(NG):
                for m in range(NB):
                    ps = pp.tile([P, TPC], FP32, tag="qkvps")
                    for k in range(KB):
                        nc.tensor.matmul(
                            ps[:],
                            lhsT=_r(w_sb[:, k * D + m * P: k * D + (m + 1) * P]),
                            rhs=_r(xt[:, k * S + ng * TPC: k * S + (ng + 1) * TPC]),
                            start=(k == 0), stop=(k == KB - 1),
                        )
                    st = stg.tile([P, TPC], FP32, tag="stage")
                    nc.vector.tensor_scalar_add(st[:], ps[:], bk_c[:, m:m + 1])
                    nc.sync.dma_start(
                        KT_t[m][:, ng * TPC:(ng + 1) * TPC], st[:])

            # --- V -> DRAM (no bias; bv folded post-softmax) ---
            w_sb = wpool.tile([P, KB * D], FP32, tag="wfull")
            for b in range(KB):
                nc.sync.dma_start(_r(w_sb[:, b * D:(b + 1) * D]),
                                  _r(wv_d.ap()[b * P:(b + 1) * P, :]))
            for mt in range(KT16):
                for nd in range(2):
                    ps = pp.tile([P, TPC], FP32, tag="qkvps")
                    for k in range(KB):
                        nc.tensor.matmul(
                            ps[:],
                            lhsT=_r(xt[:, k * S + mt * P: k * S + (mt + 1) * P]),
                            rhs=_r(w_sb[:, k * D + nd * TPC: k * D + (nd + 1) * TPC]),
                            start=(k == 0), stop=(k == KB - 1),
                        )
                    st = stg.tile([P, TPC], FP32, tag="stage")
                    nc.vector.tensor_copy(st[:], ps[:])
                    nc.sync.dma_start(
                        V_t[mt][:, nd * TPC:(nd + 1) * TPC], st[:])

        # ================= Attention phase =================
        with (
            tc.tile_pool(name="attn_sb", bufs=1) as ap_pool,
            tc.tile_pool(name="attn_kt", bufs=3) as ktp,
            tc.tile_pool(name="attn_scr", bufs=6) as axp,
            tc.tile_pool(name="attn_sps", bufs=4, space="PSUM") as sps,
            tc.tile_pool(name="attn_ops", bufs=2, space="PSUM") as ops,
            tc.tile_pool(name="attn_rbp", bufs=2, space="PSUM") as rbp,
        ):
            vp_sb = ap_pool.tile([P, KT16 * VW], FP32)   # V', 65KB
            for kt in range(KT16):
                v3 = vp_sb[:, kt * VW:(kt + 1) * VW].rearrange(
                    "p (h j) -> p h j", j=DH + 1)
                nc.scalar.copy(_r(v3[:, :, DH]), ones16[:])
                nc.sync.dma_start(
                    _r(v3[:, :, 0:DH]),
                    _r(V_t[kt][:].rearrange("p (h j) -> p h j", j=DH)))

            for hb in range(H // 2):
                kt_blk = ktp.tile([P, S], FP32, tag="ktblk")
                nc.sync.dma_start(_r(kt_blk[:]), _r(KT_t[hb][:]))
                for hpar in range(2):
                    h = 2 * hb + hpar
                    hp = hpar * DH
                    op_ps = ops.tile([DH + 1, TPC], FP32, tag="opsum")
                    for kt in range(KT16):
                        sp = sps.tile([P, TPC], FP32, tag="spsum")
                        nc.tensor.matmul(
                            sp[:],
                            lhsT=_r(kt_blk[hp:hp + DH, kt * P:(kt + 1) * P]),
                            rhs=_r(qt_sb[hp:hp + DH, hb * TPC:(hb + 1) * TPC]),
                            start=True, stop=True,
                        )
                        if use_mask:
                            mk = axp.tile([P, TPC], FP32, tag="mk")
                            nc.sync.dma_start(
                                mk[:], maskT_d.ap()[kt * P:(kt + 1) * P, :])
                            nc.vector.tensor_add(sp[:], sp[:], mk[:])
                        et = axp.tile([P, TPC], FP32, tag="expT")
                        nc.scalar.activation(_r(et[:]), sp[:], AF.Exp, scale=SCALE)
                        nc.tensor.matmul(
                            op_ps[:],
                            lhsT=_r(vp_sb[:, kt * VW + h * (DH + 1):
                                          kt * VW + (h + 1) * (DH + 1)]),
                            rhs=_r(et[:]),
                            start=(kt == 0), stop=(kt == KT16 - 1),
                        )
                    rr = axp.tile([1, TPC], FP32, tag="rrow")
                    with nc.allow_low_precision(
                            reason="fp32r tag for broadcast matmul operand"):
                        nc.vector.reciprocal(_r(rr[:]), op_ps[DH:DH + 1, :])
                    rb_ps = rbp.tile([DH, TPC], FP32, tag="rbps")
                    nc.tensor.matmul(rb_ps[:], lhsT=_r(ones_row[:]),
                                     rhs=_r(rr[:]), start=True, stop=True)
                    rb_sb = axp.tile([DH, TPC], FP32, tag="rbsb")
                    nc.scalar.copy(rb_sb[:], rb_ps[:])
                    on = axp.tile([DH, TPC], FP32, tag="onorm")
                    nc.vector.tensor_mul(on[:], op_ps[0:DH, :], rb_sb[:])
                    nc.scalar.activation(
                        _r(ot_sb[hp:hp + DH, hb * TPC:(hb + 1) * TPC]), on[:],
                        AF.Identity, bias=bv_c[hp:hp + DH, hb:hb + 1])

        # ================= Output proj + LN1 + transpose =================
        lnp = top.enter_context(tc.tile_pool(name="lnp", bufs=1))
        xln1_sb = lnp.tile([P, QT * D], FP32)      # LN1 output (natural), 16KB
        xln1T = lnp.tile([P, KB * TPC], FP32)      # its transpose, 16KB

        with (
            tc.tile_pool(name="proj_sb", bufs=1) as prp,
            tc.tile_pool(name="proj_w", bufs=2) as pwp,
            tc.tile_pool(name="proj_scr", bufs=3) as pscr,
        ):
            xblk_sb = prp.tile([P, QT * D], FP32)
            for qt in range(QT):
                nc.sync.dma_start(xblk_sb[:, qt * D:(qt + 1) * D],
                                  xblk_d.ap()[qt * P:(qt + 1) * P, :])
            bp_b = prp.tile([P, D], FP32)
            nc.sync.dma_start(bp_b[:], bp_d.ap()[None, :].to_broadcast((P, D)))
            g1_b = prp.tile([P, D], FP32)
            nc.sync.dma_start(g1_b[:], g1_d.ap()[None, :].to_broadcast((P, D)))
            be1_b = prp.tile([P, D], FP32)
            nc.sync.dma_start(be1_b[:], be1_d.ap()[None, :].to_broadcast((P, D)))

            with tc.tile_pool(name="proj_ps", bufs=8, space="PSUM") as ppp:
                pj = [[ppp.tile([P, TPC], FP32, tag="projps",
                                name=f"pj_{qt}_{nd}")
                       for nd in range(2)] for qt in range(QT)]
                for k in range(KB):
                    wpt = pwp.tile([P, D], FP32, tag="wpt")
                    nc.sync.dma_start(_r(wpt[:]), _r(wp_d.ap()[k * P:(k + 1) * P, :]))
                    for qt in range(QT):
                        for nd in range(2):
                            nc.tensor.matmul(
                                pj[qt][nd][:],
                                lhsT=_r(ot_sb[:, k * TPC + qt * P:
                                              k * TPC + (qt + 1) * P]),
                                rhs=_r(wpt[:, nd * TPC:(nd + 1) * TPC]),
                                start=(k == 0), stop=(k == KB - 1),
                            )
                for qt in range(QT):
                    y = pscr.tile([P, D], FP32, tag="y1")
                    for nd in range(2):
                        nc.vector.tensor_add(
                            y[:, nd * TPC:(nd + 1) * TPC], pj[qt][nd][:],
                            xblk_sb[:, qt * D + nd * TPC: qt * D + (nd + 1) * TPC])
                    nc.vector.tensor_add(y[:], y[:], bp_b[:])
                    _ln_chain(nc, pscr, y, xln1_sb[:, qt * D:(qt + 1) * D],
                              g1_b, be1_b, eps_t)

            with tc.tile_pool(name="tp_ps", bufs=2, space="PSUM") as tpp:
                for bd in range(NB):
                    for qt in range(QT):
                        tp = tpp.tile([P, P], FP32, tag="tps")
                        nc.tensor.transpose(
                            tp[:],
                            xln1_sb[:, qt * D + bd * P: qt * D + (bd + 1) * P],
                            ident[:])
                        nc.vector.tensor_copy(
                            _r(xln1T[:, bd * TPC + qt * P: bd * TPC + (qt + 1) * P]),
                            tp[:])

        # ================= FFN =================
        with (
            tc.tile_pool(name="ffn_sb", bufs=1) as fsb,
            tc.tile_pool(name="ffn_w", bufs=4) as fwp,
            tc.tile_pool(name="ffn_scr", bufs=2) as fscr,
        ):
            hT = fsb.tile([P, FB * TPC], FP32)    # relu(x@W1+b1)^T, 64KB
            b2_b = fsb.tile([P, D], FP32)
            nc.sync.dma_start(b2_b[:], b2_d.ap()[None, :].to_broadcast((P, D)))
            g2_b = fsb.tile([P, D], FP32)
            nc.sync.dma_start(g2_b[:], g2_d.ap()[None, :].to_broadcast((P, D)))
            be2_b = fsb.tile([P, D], FP32)
            nc.sync.dma_start(be2_b[:], be2_d.ap()[None, :].to_broadcast((P, D)))

            with tc.tile_pool(name="ffn1_ps", bufs=6, space="PSUM") as fps:
                for mf in range(FB):
                    w1t = fwp.tile([P, KB * P], FP32, tag="w1t")
                    nc.sync.dma_start(
                        _r(w1t[:].rearrange("p (k c) -> p k c", c=P)),
                        _r(w1_d.ap()[:, mf * P:(mf + 1) * P].rearrange(
                            "(k p) c -> p k c", p=P)))
                    ph = fps.tile([P, TPC], FP32, tag="fps")
                    for k in range(KB):
                        nc.tensor.matmul(
                            ph[:],
                            lhsT=_r(w1t[:, k * P:(k + 1) * P]),
                            rhs=_r(xln1T[:, k * TPC:(k + 1) * TPC]),
                            start=(k == 0), stop=(k == KB - 1),
                        )
                    nc.scalar.activation(
                        _r(hT[:, mf * TPC:(mf + 1) * TPC]), ph[:], AF.Relu,
                        bias=b1_c[:, mf:mf + 1])

            with tc.tile_pool(name="ffn2_ps", bufs=8, space="PSUM") as fp2:
                pj2 = [[fp2.tile([P, TPC], FP32, tag="f2ps",
                                 name=f"pj2_{qt}_{nd}")
                        for nd in range(2)] for qt in range(QT)]
                for k2 in range(FB):
                    w2t = fwp.tile([P, D], FP32, tag="w2t")
                    nc.sync.dma_start(_r(w2t[:]), _r(w2_d.ap()[k2 * P:(k2 + 1) * P, :]))
                    for qt in range(QT):
                        for nd in range(2):
                            nc.tensor.matmul(
                                pj2[qt][nd][:],
                                lhsT=_r(hT[:, k2 * TPC + qt * P:
                                           k2 * TPC + (qt + 1) * P]),
                                rhs=_r(w2t[:, nd * TPC:(nd + 1) * TPC]),
                                start=(k2 == 0), stop=(k2 == FB - 1),
                            )
                for qt in range(QT):
                    y2 = fscr.tile([P, D], FP32, tag="y2")
                    for nd in range(2):
                        nc.vector.tensor_add(
                            y2[:, nd * TPC:(nd + 1) * TPC], pj2[qt][nd][:],
                            xln1_sb[:, qt * D + nd * TPC: qt * D + (nd + 1) * TPC])
                    nc.vector.tensor_add(y2[:], y2[:], b2_b[:])
                    yo = fscr.tile([P, D], FP32, tag="yo")
                    _ln_chain(nc, fscr, y2, yo[:], g2_b, be2_b, eps_t)
                    nc.sync.dma_start(out_d.ap()[qt * P:(qt + 1) * P, :], yo[:])


_PROG_CACHE: dict = {}


def _get_program(use_mask: bool) -> bass.Bass:
    if use_mask not in _PROG_CACHE:
        _PROG_CACHE[use_mask] = build_program(use_mask)
    return _PROG_CACHE[use_mask]


def make_in_maps(x, mask, Wq, bq, Wk, bk, Wv, bv, Wp, bp,
                 gamma1, beta1, W1, b1, W2, b2, gamma2, beta2):
    x = np.asarray(x, np.float32)
    mask = np.asarray(mask)
    use_mask = not bool(mask.all())
    common = {
        "wq": np.ascontiguousarray(Wq, np.float32),
        "wk": np.ascontiguousarray(Wk, np.float32),
        "wv": np.ascontiguousarray(Wv, np.float32),
        "wp": np.ascontiguousarray(Wp, np.float32),
        "w1": np.ascontiguousarray(W1, np.float32),
        "w2": np.ascontiguousarray(W2, np.float32),
        "bq": np.ascontiguousarray(bq, np.float32),
        "bk": np.ascontiguousarray(bk, np.float32),
        "bv": np.ascontiguousarray(bv, np.float32),
        "bp": np.ascontiguousarray(bp, np.float32),
        "b1": np.ascontiguousarray(b1, np.float32),
        "b2": np.ascontiguousarray(b2, np.float32),
        "g1": np.ascontiguousarray(gamma1, np.float32),
        "be1": np.ascontiguousarray(beta1, np.float32),
        "g2": np.ascontiguousarray(gamma2, np.float32),
        "be2": np.ascontiguousarray(beta2, np.float32),
    }
    if use_mask:
        mbias = np.where(mask, np.float32(0.0), np.float32(-1e12)).astype(np.float32)
    in_maps = []
    for c in range(N_CORES):
        b, j = divmod(c, 4)
        xb = x[b]
        m = dict(common)
        m["xT"] = np.ascontiguousarray(xb.T)
        m["xTq"] = np.ascontiguousarray(xb[j * TPC:(j + 1) * TPC].T)
        m["xblk"] = np.ascontiguousarray(xb[j * TPC:(j + 1) * TPC])
        if use_mask:
            m["maskT"] = np.ascontiguousarray(mbias.T[:, j * TPC:(j + 1) * TPC])
        in_maps.append(m)
    return use_mask, in_maps


def assemble_output(results) -> np.ndarray:
    out = np.empty((2, S, D), np.float32)
    for c in range(N_CORES):
        b, j = divmod(c, 4)
        out[b, j * TPC:(j + 1) * TPC] = results[c]["out"]
    return out


def kernel(**inputs) -> np.ndarray:
    use_mask, in_maps = make_in_maps(**inputs)
    nc = _get_program(use_mask)
    res = run_bass_kernel_spmd(nc, in_maps, list(range(N_CORES)))
    return assemble_output(res.results)



# revision 13
# speedup vs baseline: 4.0720x; 4.0720x over previous
"""Trainium2 Bass kernel for a dense transformer encoder layer.

Problem (hardcoded): x [2, 2048, 1024], 16 heads, FFN 4096, fp32,
post-LN residual blocks, mask additively applied before softmax.

Sharding: sequence-parallel over the 4096 tokens -> 512 tokens per core
(cores 0-3 handle batch 0, cores 4-7 batch 1). Every core computes the
full-batch K/V projections itself, keeps K^T / V' entirely in SBUF
(bf16), then runs attention for its own 512 queries, the output
projection (folded into the attention head loop via an SBUF
accumulator), LN1, the FFN and LN2.

All matmul operands are bf16 (same PE rate as fp32r at free-dim 512 but
half the DMA/SBUF traffic and fast weight loads); accumulation is fp32
in PSUM. Softmax/LN arithmetic stays fp32.

Matmul layouts (out = lhsT.T @ rhs, contraction on the partition dim):
  Q^T/K^T : lhsT = W k/m-tile [din,dout], rhs = x^T [din,tok]  -> [dout,tok]
  V       : lhsT = x^T [din,tok],  rhs = Wv [din,dout]         -> [tok,dout]
  scoresT : lhsT = K^T head [dh,kpos], rhs = Q^T head [dh,qpos]-> [kpos,qpos]
            (head pairs run concurrently in disjoint PE row groups)
  attn@V' : lhsT = V' [kpos,dh+1], rhs = expT [kpos,qpos]      -> [dh+1,qpos]
            (V' has a ones column -> row dh is the softmax denominator)
  outproj : lhsT = o^T [din,q], rhs = Wp [din,dout]            -> [q,dout]
  FFN1    : lhsT = W1 [din,dffn], rhs = xln1^T [din,q]         -> [dffn,q]
  FFN2    : lhsT = h^T [dffn,q], rhs = W2 [dffn,dout]          -> [q,dout]

Host-side exact folds: bp' = bp + bv @ Wp into the residual (attention
with V-bias == attention without + bv once rows sum to 1), so bv/bp
never touch the device.
"""

from contextlib import ExitStack

import numpy as np

import concourse.bass as bass
import concourse.mybir as mybir
import concourse.tile as tile
from concourse.bass_utils import run_bass_kernel_spmd
from concourse.masks import make_identity
from concourse.vector_clock import ScopedClock

FP32 = mybir.dt.float32
FP32R = mybir.dt.float32r
BF16 = mybir.dt.bfloat16
AF = mybir.ActivationFunctionType
ALU = mybir.AluOpType

P = 128
D = 1024
F = 4096
H = 16
DH = 64
S = 2048          # tokens per batch
TPC = 512         # tokens (queries) per core
NB = D // P       # 8 dout blocks
KB = D // P       # 8 contraction tiles over D
FB = F // P       # 32 dffn tiles
QT = TPC // P     # 4 query tiles
KT16 = S // P     # 16 kpos tiles
NG = S // TPC     # 4 kpos 512-slices
VW = H * (DH + 1)  # 1040: V' row width
SCALE = DH ** -0.5
EPS = 1e-6
N_CORES = 8


# --- Tile tail-drain fix: this walrus build allows only one sem-wait per
# instruction; Tile's final drain accumulates several. Split them across
# dedicated nops before draining.
def _patched_drain_and_barrier(self, tick_clock, wait_clock):
    probe = self.nc.sync.nop(nofuse=True, hint="drain_wait_split")
    wait_clock.add_sem_waits(probe.ins, ScopedClock({None: tick_clock.global_clock}))
    si = probe.ins.sync_info
    if si is not None and si.on_wait and len(si.on_wait) > 1:
        waits = list(si.on_wait)
        si.on_wait = waits[:1]
        for w in waits[1:]:
            extra = self.nc.sync.nop(nofuse=True, hint="drain_wait_split")
            esi = extra.ins.sync_info
            if esi is None:
                extra.ins.sync_info = mybir.SyncInfo(on_wait=[w], on_update=[])
            else:
                esi.on_wait = [w]
    self.nc.sync.drain()
    self.nc.all_engine_barrier()
    assert self.sems is not None
    popped = self.nc._tile_sem_poison_stack.pop()
    assert popped is self._sem_poison
    self.nc.clear_and_free_semaphores(list(self.sems.allocated().values()))
    self.nc.all_engine_barrier()


if getattr(tile.TileContext, "_drain_patch", None) is None:
    tile.TileContext._drain_and_barrier = _patched_drain_and_barrier
    tile.TileContext._drain_patch = True


def _r(ap):
    return ap.bitcast(FP32R)


def _split_waits(nc):
    """Walrus codegen accepts at most one sem-wait per instruction (two on
    EventSemaphore). Tile's scheduler can emit more; hoist the surplus onto
    same-engine EventSemaphore instructions inserted just before."""
    uid = [0]
    for bb in nc.m.functions[0].blocks:
        new_insts = []
        for inst in bb.instructions:
            si = inst.sync_info
            limit = 2 if isinstance(inst, mybir.InstEventSemaphore) else 1
            if si is not None and si.on_wait and len(si.on_wait) > limit:
                waits = list(si.on_wait)
                extra, keep = waits[:-limit], waits[-limit:]
                for i in range(0, len(extra), 2):
                    uid[0] += 1
                    ev = mybir.InstEventSemaphore(
                        name=f"I-wsplit-{uid[0]}",
                        engine=inst.engine,
                        sync_info=mybir.SyncInfo(
                            on_wait=extra[i:i + 2], on_update=[]),
                    )
                    nc.register_instruction(ev)
                    new_insts.append(ev)
                si.on_wait = keep
            new_insts.append(inst)
        if len(new_insts) != len(bb.instructions):
            bb.instructions[:] = new_insts


def _ln_chain(nc, pool, y, s1, out_ap, gamma_b, beta_b):
    """LayerNorm over the free dim of y [128, D] (torch semantics: unbiased
    std, denominator std + eps), given s1 = row-sums of y. Uses
    var = (E[y^2]*D - D*mean^2)/(D-1) so the Square pass runs concurrently
    with the mean computation. Writes out_ap (any dtype)."""
    sq = pool.tile([P, D], FP32, tag="ln_sq")
    ss2 = pool.tile([P, 1], FP32, tag="ln_ss2")
    nc.scalar.activation(sq[:], y[:], AF.Square, accum_out=ss2[:])
    mn = pool.tile([P, 1], FP32, tag="ln_mn")
    nc.scalar.mul(mn[:], s1[:], 1.0 / D)
    mn2 = pool.tile([P, 1], FP32, tag="ln_mn2")
    nc.vector.tensor_scalar_mul(mn2[:], mn[:], mn[:])
    var = pool.tile([P, 1], FP32, tag="ln_var")
    nc.vector.scalar_tensor_tensor(
        var[:], mn2[:], -float(D), ss2[:], op0=ALU.mult, op1=ALU.add)
    std = pool.tile([P, 1], FP32, tag="ln_std")
    nc.scalar.activation(std[:], var[:], AF.Sqrt, scale=1.0 / (D - 1))
    nc.vector.tensor_scalar_add(std[:], std[:], EPS)
    rcp = pool.tile([P, 1], FP32, tag="ln_rcp")
    nc.vector.reciprocal(rcp[:], std[:])
    t1 = pool.tile([P, D], FP32, tag="ln_t1")
    nc.vector.scalar_tensor_tensor(
        t1[:], y[:], mn[:], gamma_b[:], op0=ALU.subtract, op1=ALU.mult)
    nc.vector.scalar_tensor_tensor(
        out_ap, t1[:], rcp[:], beta_b[:], op0=ALU.mult, op1=ALU.add)


def build_program(use_mask: bool) -> bass.Bass:
    nc = bass.Bass(target_bir_lowering=False, debug=False)

    # ---- I/O ----
    xT_d = nc.dram_tensor("xT", [D, S], BF16, kind="ExternalInput")
    xTq_d = nc.dram_tensor("xTq", [D, TPC], BF16, kind="ExternalInput")
    xres_d = nc.dram_tensor("xres", [TPC, D], FP32, kind="ExternalInput")
    wq_d = nc.dram_tensor("wq", [D, D], BF16, kind="ExternalInput")
    wk_d = nc.dram_tensor("wk", [D, D], BF16, kind="ExternalInput")
    wv_d = nc.dram_tensor("wv", [D, D], BF16, kind="ExternalInput")
    wp_d = nc.dram_tensor("wp", [D, D], BF16, kind="ExternalInput")
    w1_d = nc.dram_tensor("w1", [D, F], BF16, kind="ExternalInput")
    w2_d = nc.dram_tensor("w2", [F, D], BF16, kind="ExternalInput")
    bq_d = nc.dram_tensor("bq", [D], FP32, kind="ExternalInput")
    bk_d = nc.dram_tensor("bk", [D], FP32, kind="ExternalInput")
    b1_d = nc.dram_tensor("b1", [F], FP32, kind="ExternalInput")
    b2_d = nc.dram_tensor("b2", [D], FP32, kind="ExternalInput")
    g1_d = nc.dram_tensor("g1", [D], FP32, kind="ExternalInput")
    be1_d = nc.dram_tensor("be1", [D], FP32, kind="ExternalInput")
    g2_d = nc.dram_tensor("g2", [D], FP32, kind="ExternalInput")
    be2_d = nc.dram_tensor("be2", [D], FP32, kind="ExternalInput")
    if use_mask:
        maskT_d = nc.dram_tensor("maskT", [S, TPC], FP32, kind="ExternalInput")
    out_d = nc.dram_tensor("out", [TPC, D], FP32, kind="ExternalOutput")

    with tile.TileContext(nc) as tc:
        _build_body(
            nc, tc, use_mask,
            xT_d, xTq_d, xres_d, wq_d, wk_d, wv_d, wp_d, w1_d, w2_d,
            bq_d, bk_d, b1_d, b2_d, g1_d, be1_d, g2_d, be2_d,
            maskT_d if use_mask else None, out_d,
        )
    _split_waits(nc)
    return nc


def _build_body(nc, tc, use_mask, xT_d, xTq_d, xres_d, wq_d, wk_d, wv_d,
                wp_d, w1_d, w2_d, bq_d, bk_d, b1_d, b2_d, g1_d, be1_d,
                g2_d, be2_d, maskT_d, out_d):
    with ExitStack() as top:
        consts = top.enter_context(tc.tile_pool(name="consts", bufs=1))
        ident = consts.tile([P, P], BF16)
        make_identity(nc, ident[:])
        bq_c = consts.tile([P, NB], FP32)
        nc.sync.dma_start(bq_c[:], bq_d.ap().rearrange("(b p) -> p b", p=P))
        bk_c = consts.tile([P, NB], FP32)
        nc.sync.dma_start(bk_c[:], bk_d.ap().rearrange("(b p) -> p b", p=P))
        b1_c = consts.tile([P, FB], FP32)
        nc.sync.dma_start(b1_c[:], b1_d.ap().rearrange("(b p) -> p b", p=P))
        ones_row = consts.tile([1, DH], BF16)
        nc.vector.memset(ones_row[:], 1.0)

        persist = top.enter_context(tc.tile_pool(name="persist", bufs=1))
        pacc = persist.tile([P, QT * D], BF16)       # proj accumulator, 8KB

        # ================= QKV + attention =================
        with ExitStack() as mid:
            attn_sb = mid.enter_context(tc.tile_pool(name="attn_sb", bufs=1))
            qt_sb = attn_sb.tile([P, NB * TPC], BF16)   # Q^T (+bq), 8KB/part
            kt_sb = attn_sb.tile([P, NB * S], BF16)     # K^T (+bk), 32KB/part
            vp_sb = attn_sb.tile([P, KT16 * VW], BF16)  # V', 32.5KB/part
            ot_sb = attn_sb.tile([P, NB * TPC], BF16)   # o^T normalized, 8KB

            # V' ones columns (softmax denominator trick)
            for kt in range(KT16):
                v3 = vp_sb[:, kt * VW:(kt + 1) * VW].rearrange(
                    "p (h j) -> p h j", j=DH + 1)
                nc.vector.memset(v3[:, :, DH:DH + 1], 1.0)

            xp = mid.enter_context(tc.tile_pool(name="xt", bufs=1))
            xt = xp.tile([P, KB * S], BF16)          # x^T full batch, 32KB
            for b in range(KB):
                nc.sync.dma_start(xt[:, b * S:(b + 1) * S],
                                  xT_d.ap()[b * P:(b + 1) * P, :])
            wpool = mid.enter_context(tc.tile_pool(name="qkv_w", bufs=2))
            qkv_ps = mid.enter_context(
                tc.tile_pool(name="qkv_ps", bufs=2, space="PSUM"))

            # --- Q^T -> qt_sb (+bq) --- own 512 query columns only
            xtq = xp.tile([P, KB * TPC], BF16)
            for b in range(KB):
                nc.sync.dma_start(xtq[:, b * TPC:(b + 1) * TPC],
                                  xTq_d.ap()[b * P:(b + 1) * P, :])
            w_sb = wpool.tile([P, KB * D], BF16, tag="w")
            for b in range(KB):
                nc.sync.dma_start(w_sb[:, b * D:(b + 1) * D],
                                  wq_d.ap()[b * P:(b + 1) * P, :])
            for m in range(NB):
                ps = qkv_ps.tile([P, TPC], FP32, tag="qkvps")
                for k in range(KB):
                    nc.tensor.matmul(
                        ps[:],
                        lhsT=w_sb[:, k * D + m * P: k * D + (m + 1) * P],
                        rhs=xtq[:, k * TPC:(k + 1) * TPC],
                        start=(k == 0), stop=(k == KB - 1),
                    )
                nc.vector.tensor_scalar_add(
                    qt_sb[:, m * TPC:(m + 1) * TPC], ps[:], bq_c[:, m:m + 1])

            # --- K^T -> kt_sb (+bk), head-pair-major ---
            w_sb = wpool.tile([P, KB * D], BF16, tag="w")
            for b in range(KB):
                nc.sync.dma_start(w_sb[:, b * D:(b + 1) * D],
                                  wk_d.ap()[b * P:(b + 1) * P, :])
            for m in range(NB):
                for ng in range(NG):
                    ps = qkv_ps.tile([P, TPC], FP32, tag="qkvps")
                    for k in range(KB):
                        nc.tensor.matmul(
                            ps[:],
                            lhsT=w_sb[:, k * D + m * P: k * D + (m + 1) * P],
                            rhs=xt[:, k * S + ng * TPC: k * S + (ng + 1) * TPC],
                            start=(k == 0), stop=(k == KB - 1),
                        )
                    nc.vector.tensor_scalar_add(
                        kt_sb[:, m * S + ng * TPC: m * S + (ng + 1) * TPC],
                        ps[:], bk_c[:, m:m + 1])

            # --- V -> vp_sb (no bias; bv folded into xres on host) ---
            w_sb = wpool.tile([P, KB * D], BF16, tag="w")
            for b in range(KB):
                nc.sync.dma_start(w_sb[:, b * D:(b + 1) * D],
                                  wv_d.ap()[b * P:(b + 1) * P, :])
            for mt in range(KT16):
                v3 = vp_sb[:, mt * VW:(mt + 1) * VW].rearrange(
                    "p (h j) -> p h j", j=DH + 1)
                for nd in range(2):
                    ps = qkv_ps.tile([P, TPC], FP32, tag="qkvps")
                    for k in range(KB):
                        nc.tensor.matmul(
                            ps[:],
                            lhsT=xt[:, k * S + mt * P: k * S + (mt + 1) * P],
                            rhs=w_sb[:, k * D + nd * TPC: k * D + (nd + 1) * TPC],
                            start=(k == 0), stop=(k == KB - 1),
                        )
                    nc.vector.tensor_copy(
                        v3[:, nd * 8:(nd + 1) * 8, 0:DH],
                        ps[:].rearrange("p (h j) -> p h j", j=DH))

            # --- attention + output projection ---
            wpp = mid.enter_context(tc.tile_pool(name="wp_sb", bufs=1))
            wp_sb = wpp.tile([P, KB * D], BF16)
            for b in range(KB):
                nc.sync.dma_start(wp_sb[:, b * D:(b + 1) * D],
                                  wp_d.ap()[b * P:(b + 1) * P, :])

            sp2p = mid.enter_context(
                tc.tile_pool(name="sp2", bufs=1, space="PSUM"))
            opp = mid.enter_context(
                tc.tile_pool(name="opps", bufs=2, space="PSUM"))
            rbpj = mid.enter_context(
                tc.tile_pool(name="rbpj", bufs=2, space="PSUM"))
            etp = mid.enter_context(tc.tile_pool(name="et", bufs=4))
            scr = mid.enter_context(tc.tile_pool(name="attn_scr", bufs=2))
            if use_mask:
                mkp = mid.enter_context(tc.tile_pool(name="mk", bufs=3))

            for hb in range(NB):
                ops = [opp.tile([DH + 1, TPC], FP32, tag="op",
                                name=f"op_{hb}_{i}") for i in range(2)]
                for kt in range(KT16):
                    sp = sp2p.tile([P, 2 * TPC], FP32, tag="sp")
                    for hpar in range(2):
                        hp = hpar * DH
                        nc.tensor.matmul(
                            sp[:, hpar * TPC:(hpar + 1) * TPC],
                            lhsT=kt_sb[hp:hp + DH,
                                       hb * S + kt * P: hb * S + (kt + 1) * P],
                            rhs=qt_sb[hp:hp + DH, hb * TPC:(hb + 1) * TPC],
                            start=True, stop=True,
                        )
                    if use_mask:
                        mk = mkp.tile([P, TPC], FP32, tag="mk")
                        nc.sync.dma_start(
                            mk[:], maskT_d.ap()[kt * P:(kt + 1) * P, :])
                        for hpar in range(2):
                            nc.vector.tensor_add(
                                sp[:, hpar * TPC:(hpar + 1) * TPC],
                                sp[:, hpar * TPC:(hpar + 1) * TPC], mk[:])
                    et = etp.tile([P, 2 * TPC], BF16, tag="et")
                    nc.scalar.activation(et[:], sp[:], AF.Exp, scale=SCALE)
                    for hpar in range(2):
                        h = 2 * hb + hpar
                        nc.tensor.matmul(
                            ops[hpar][:],
                            lhsT=vp_sb[:, kt * VW + h * (DH + 1):
                                       kt * VW + (h + 1) * (DH + 1)],
                            rhs=et[:, hpar * TPC:(hpar + 1) * TPC],
                            start=(kt == 0), stop=(kt == KT16 - 1),
                        )
                for hpar in range(2):
                    hp = hpar * DH
                    rr = scr.tile([1, TPC], FP32, tag="rr")
                    nc.vector.reciprocal(rr[:], ops[hpar][DH:DH + 1, :])
                    rrb = scr.tile([1, TPC], BF16, tag="rrb")
                    nc.vector.tensor_copy(rrb[:], rr[:])
                    rb_ps = rbpj.tile([DH, TPC], FP32, tag="rbpj")
                    nc.tensor.matmul(rb_ps[:], lhsT=ones_row[:],
                                     rhs=rrb[:], start=True, stop=True)
                    rb_sb = scr.tile([DH, TPC], BF16, tag="rbsb")
                    nc.vector.tensor_copy(rb_sb[:], rb_ps[:])
                    nc.vector.tensor_mul(
                        ot_sb[hp:hp + DH, hb * TPC:(hb + 1) * TPC],
                        ops[hpar][0:DH, :], rb_sb[:])
                # output projection for this head pair -> pacc
                for qt in range(QT):
                    for nd in range(2):
                        pj = rbpj.tile([P, TPC], FP32, tag="rbpj")
                        nc.tensor.matmul(
                            pj[:],
                            lhsT=ot_sb[:, hb * TPC + qt * P:
                                       hb * TPC + (qt + 1) * P],
                            rhs=wp_sb[:, hb * D + nd * TPC:
                                      hb * D + (nd + 1) * TPC],
                            start=True, stop=True,
                        )
                        dst = pacc[:, qt * D + nd * TPC: qt * D + (nd + 1) * TPC]
                        if hb == 0:
                            nc.vector.tensor_copy(dst, pj[:])
                        else:
                            nc.vector.tensor_add(dst, dst, pj[:])

        # ================= LN1 + transpose + FFN + LN2 =================
        with ExitStack() as tail:
            big = tail.enter_context(tc.tile_pool(name="tail_big", bufs=1))
            xres_sb = big.tile([P, QT * D], FP32)    # residual + bp + bv@Wp
            for qt in range(QT):
                nc.sync.dma_start(xres_sb[:, qt * D:(qt + 1) * D],
                                  xres_d.ap()[qt * P:(qt + 1) * P, :])
            xln1 = big.tile([P, QT * D], BF16)       # LN1 output (natural)
            xln1T = big.tile([P, KB * TPC], BF16)    # its transpose
            hT = big.tile([P, FB * TPC], BF16)       # relu(x@W1+b1)^T, 32KB
            b2_b = big.tile([P, D], FP32)
            nc.sync.dma_start(b2_b[:], b2_d.ap()[None, :].to_broadcast((P, D)))
            g1_b = big.tile([P, D], FP32)
            nc.sync.dma_start(g1_b[:], g1_d.ap()[None, :].to_broadcast((P, D)))
            be1_b = big.tile([P, D], FP32)
            nc.sync.dma_start(be1_b[:], be1_d.ap()[None, :].to_broadcast((P, D)))
            g2_b = big.tile([P, D], FP32)
            nc.sync.dma_start(g2_b[:], g2_d.ap()[None, :].to_broadcast((P, D)))
            be2_b = big.tile([P, D], FP32)
            nc.sync.dma_start(be2_b[:], be2_d.ap()[None, :].to_broadcast((P, D)))

            lnp = tail.enter_context(tc.tile_pool(name="ln_scr", bufs=2))
            with tc.tile_pool(name="tp_ps", bufs=2, space="PSUM") as tpp:
                for qt in range(QT):
                    y = lnp.tile([P, D], FP32, tag="ln_y")
                    s1 = lnp.tile([P, 1], FP32, tag="ln_s1")
                    nc.vector.scalar_tensor_tensor(
                        y[:], pacc[:, qt * D:(qt + 1) * D], 0.0,
                        xres_sb[:, qt * D:(qt + 1) * D],
                        op0=ALU.add, op1=ALU.add, accum_out=s1[:])
                    _ln_chain(nc, lnp, y, s1,
                              xln1[:, qt * D:(qt + 1) * D], g1_b, be1_b)
                    for bd in range(NB):
                        tp = tpp.tile([P, P], BF16, tag="tps")
                        nc.tensor.transpose(
                            tp[:],
                            xln1[:, qt * D + bd * P: qt * D + (bd + 1) * P],
                            ident[:])
                        nc.vector.tensor_copy(
                            xln1T[:, bd * TPC + qt * P: bd * TPC + (qt + 1) * P],
                            tp[:])

            fwp = tail.enter_context(tc.tile_pool(name="ffn_w", bufs=4))
            with tc.tile_pool(name="ffn1_ps", bufs=4, space="PSUM") as fps:
                for mf in range(FB):
                    w1t = fwp.tile([P, KB * P], BF16, tag="w1t")
                    nc.sync.dma_start(
                        w1t[:].rearrange("p (k c) -> p k c", c=P),
                        w1_d.ap()[:, mf * P:(mf + 1) * P].rearrange(
                            "(k p) c -> p k c", p=P))
                    ph = fps.tile([P, TPC], FP32, tag="fps")
                    for k in range(KB):
                        nc.tensor.matmul(
                            ph[:],
                            lhsT=w1t[:, k * P:(k + 1) * P],
                            rhs=xln1T[:, k * TPC:(k + 1) * TPC],
                            start=(k == 0), stop=(k == KB - 1),
                        )
                    nc.scalar.activation(
                        hT[:, mf * TPC:(mf + 1) * TPC], ph[:], AF.Relu,
                        bias=b1_c[:, mf:mf + 1])

            with tc.tile_pool(name="ffn2_ps", bufs=8, space="PSUM") as fp2:
                pj2 = [[fp2.tile([P, TPC], FP32, tag="f2ps",
                                 name=f"pj2_{qt}_{nd}")
                        for nd in range(2)] for qt in range(QT)]
                for k2 in range(FB):
                    w2t = fwp.tile([P, D], BF16, tag="w2t")
                    nc.sync.dma_start(w2t[:], w2_d.ap()[k2 * P:(k2 + 1) * P, :])
                    for qt in range(QT):
                        for nd in range(2):
                            nc.tensor.matmul(
                                pj2[qt][nd][:],
                                lhsT=hT[:, k2 * TPC + qt * P:
                                        k2 * TPC + (qt + 1) * P],
                                rhs=w2t[:, nd * TPC:(nd + 1) * TPC],
                                start=(k2 == 0), stop=(k2 == FB - 1),
                            )
                for qt in range(QT):
                    y2 = lnp.tile([P, D], FP32, tag="ln_y")
                    for nd in range(2):
                        nc.vector.tensor_add(
                            y2[:, nd * TPC:(nd + 1) * TPC], pj2[qt][nd][:],
                            xln1[:, qt * D + nd * TPC: qt * D + (nd + 1) * TPC])
                    s1 = lnp.tile([P, 1], FP32, tag="ln_s1")
                    nc.vector.scalar_tensor_tensor(
                        y2[:], y2[:], 0.0, b2_b[:],
                        op0=ALU.add, op1=ALU.add, accum_out=s1[:])
                    yo = lnp.tile([P, D], FP32, tag="ln_yo")
                    _ln_chain(nc, lnp, y2, s1, yo[:], g2_b, be2_b)
                    nc.sync.dma_start(out_d.ap()[qt * P:(qt + 1) * P, :], yo[:])


_PROG_CACHE: dict = {}


def _get_program(use_mask: bool) -> bass.Bass:
    if use_mask not in _PROG_CACHE:
        _PROG_CACHE[use_mask] = build_program(use_mask)
    return _PROG_CACHE[use_mask]


def make_in_maps(x, mask, Wq, bq, Wk, bk, Wv, bv, Wp, bp,
                 gamma1, beta1, W1, b1, W2, b2, gamma2, beta2):
    import ml_dtypes
    BF = ml_dtypes.bfloat16

    x = np.asarray(x, np.float32)
    mask = np.asarray(mask)
    use_mask = not bool(mask.all())
    Wp32 = np.ascontiguousarray(Wp, np.float32)
    # exact fold: attention(V + bv) == attention(V) + bv (softmax rows sum
    # to 1), so o@Wp + bp == o_nobias@Wp + (bv@Wp + bp); fold into residual.
    res_bias = (np.asarray(bv, np.float32) @ Wp32
                + np.asarray(bp, np.float32)).astype(np.float32)
    common = {
        "wq": np.ascontiguousarray(Wq).astype(BF),
        "wk": np.ascontiguousarray(Wk).astype(BF),
        "wv": np.ascontiguousarray(Wv).astype(BF),
        "wp": Wp32.astype(BF),
        "w1": np.ascontiguousarray(W1).astype(BF),
        "w2": np.ascontiguousarray(W2).astype(BF),
        "bq": np.ascontiguousarray(bq, np.float32),
        "bk": np.ascontiguousarray(bk, np.float32),
        "b1": np.ascontiguousarray(b1, np.float32),
        "b2": np.ascontiguousarray(b2, np.float32),
        "g1": np.ascontiguousarray(gamma1, np.float32),
        "be1": np.ascontiguousarray(beta1, np.float32),
        "g2": np.ascontiguousarray(gamma2, np.float32),
        "be2": np.ascontiguousarray(beta2, np.float32),
    }
    if use_mask:
        mbias = np.where(mask, np.float32(0.0), np.float32(-1e12)).astype(np.float32)
    in_maps = []
    for c in range(N_CORES):
        b, j = divmod(c, 4)
        xb = x[b]
        m = dict(common)
        m["xT"] = np.ascontiguousarray(xb.T).astype(BF)
        m["xTq"] = np.ascontiguousarray(xb[j * TPC:(j + 1) * TPC].T).astype(BF)
        m["xres"] = np.ascontiguousarray(
            xb[j * TPC:(j + 1) * TPC] + res_bias[None, :])
        if use_mask:
            m["maskT"] = np.ascontiguousarray(mbias.T[:, j * TPC:(j + 1) * TPC])
        in_maps.append(m)
    return use_mask, in_maps


def assemble_output(results) -> np.ndarray:
    out = np.empty((2, S, D), np.float32)
    for c in range(N_CORES):
        b, j = divmod(c, 4)
        out[b, j * TPC:(j + 1) * TPC] = results[c]["out"]
    return out


def kernel(**inputs) -> np.ndarray:
    use_mask, in_maps = make_in_maps(**inputs)
    nc = _get_program(use_mask)
    res = run_bass_kernel_spmd(nc, in_maps, list(range(N_CORES)))
    return assemble_output(res.results)
